# revision 8
# baseline (speedup 1.0000x reference)
"""Trainium2 Bass kernel for batched multi-head attention (nn_Attend).

Inputs q, k, v: [B=4, H=16, D=64, N=2048] fp32, layout (b, h, d, n).
  sim  = einsum('bhdi,bhdj->bhij', q, k) / sqrt(D)
  attn = softmax(sim, axis=-1)
  out  = einsum('bhij,bhdj->bhdi', attn, v)

Sharding: B*H = 64 heads are embarrassingly parallel; 8 heads per NeuronCore
across 8 cores.

Per-core kernel, per head:
  simT[j,i] = sum_d k[d,j] q[d,i]        PE matmul, lhsT=k-chunk, f32r full rate
  expT[j,i] = exp(simT / 8)              ACT (scale fused into activation),
                                         PSUM -> SBUF bf16
  acc[d,i], s[i] = [vT | 1]^T expT       PE matmul accumulated over j-chunks;
                                         the appended ones column yields the
                                         softmax denominator s[i] for free
  out[d,i] = acc[d,i] / s[i]             DVE reciprocal + DMA partition
                                         broadcast + DVE multiply

Softmax max-subtraction is skipped: logits are ~N(0,1) (|sim| < ~6), so
exp() stays far from fp32/bf16 range limits and softmax is shift-invariant.
"""

import numpy as np

import concourse.bacc as bacc
import concourse.bass as bass  # noqa: F401
import concourse.mybir as mybir
import concourse.tile as tile
from concourse.masks import make_identity

B, H, D, N = 4, 16, 64, 2048
NCORES = 8
HPC = (B * H) // NCORES  # heads per core = 8
NJC = N // 128           # j-chunks per head = 16
NIP = N // 512           # 512-wide i-pieces = 4
SCALE = float(D) ** -0.5


def _build_bass():
    nc = bacc.Bacc()
    f32 = mybir.dt.float32
    f32r = mybir.dt.float32r
    bf16 = mybir.dt.bfloat16

    q_d = nc.declare_dram_parameter("q", [HPC, D, N], f32r, isOutput=False)
    k_d = nc.declare_dram_parameter("k", [HPC, D, N], f32r, isOutput=False)
    v_d = nc.declare_dram_parameter("v", [HPC, D, N], f32, isOutput=False)
    out_d = nc.declare_dram_parameter("out", [HPC, D, N], f32, isOutput=True)

    with tile.TileContext(nc) as tc:
        const_pool = tc.alloc_tile_pool(name="const", bufs=1)
        ident = const_pool.tile([D, D], f32, name="ident")
        make_identity(nc, ident)

        # vT[:, h, jc, 0:64] = v[h][:, jc*128:(jc+1)*128].T ; vT[:, h, jc, 64] = 1
        vT = const_pool.tile([128, HPC, NJC, D + 1], bf16, name="vT")
        nc.vector.memset(vT[:, :, :, D : D + 1], 1.0)

        with (
            tc.tile_pool(name="vload", bufs=2) as vload_pool,
            tc.tile_pool(name="qk", bufs=2) as qk_pool,
            tc.tile_pool(name="expt", bufs=6) as expt_pool,
            tc.tile_pool(name="simps", bufs=2, space="PSUM") as sim_pool,
            tc.tile_pool(name="avps", bufs=4, space="PSUM") as av_pool,
            tc.tile_pool(name="outsb", bufs=2) as out_pool,
            tc.tile_pool(name="norm", bufs=2) as norm_pool,
            tc.tile_pool(name="dramscratch", bufs=2, space="DRAM") as dram_pool,
        ):
            # ---- Prologue: transpose v for every head into vT (PE transpose,
            # batched 8 blocks per PSUM tile so the PSUM->SBUF copy is wide).
            for h in range(HPC):
                v_sb = vload_pool.tile([D, N], f32, tag="v")
                nc.sync.dma_start(out=v_sb, in_=v_d[h])
                for grp in range(2):
                    tp = av_pool.tile([128, 512], f32, tag="av")
                    for t in range(8):
                        jc = grp * 8 + t
                        nc.tensor.transpose(
                            tp[:, t * D : (t + 1) * D],
                            v_sb[:, jc * 128 : (jc + 1) * 128],
                            ident,
                        )
                    nc.vector.tensor_copy(
                        out=vT[:, h, grp * 8 : (grp + 1) * 8, 0:D],
                        in_=tp.rearrange("p (t c) -> p t c", t=8),
                    )

            # ---- Main loop over this core's heads.
            for h in range(HPC):
                q_sb = qk_pool.tile([D, N], f32r, tag="q")
                k_sb = qk_pool.tile([D, N], f32r, tag="k")
                nc.sync.dma_start(out=q_sb, in_=q_d[h])
                nc.sync.dma_start(out=k_sb, in_=k_d[h])

                av = [
                    av_pool.tile([128, 512], f32, tag="av", name=f"av{ip}")
                    for ip in range(NIP)
                ]

                for jc in range(NJC):
                    expT = expt_pool.tile([128, N], bf16, tag="expT")
                    for half in range(2):
                        sim = sim_pool.tile([128, 1024], f32, tag="sim")
                        for s2 in range(2):
                            i0 = half * 1024 + s2 * 512
                            nc.tensor.matmul(
                                sim[:, s2 * 512 : (s2 + 1) * 512],
                                lhsT=k_sb[:, jc * 128 : (jc + 1) * 128],
                                rhs=q_sb[:, i0 : i0 + 512],
                                start=True,
                                stop=True,
                                skip_group_check=True,
                            )
                        nc.scalar.activation(
                            out=expT[:, half * 1024 : (half + 1) * 1024],
                            in_=sim[:, :],
                            func=mybir.ActivationFunctionType.Exp,
                            scale=SCALE,
                        )
                    for ip in range(NIP):
                        nc.tensor.matmul(
                            av[ip][0 : D + 1, :],
                            lhsT=vT[:, h, jc, :],
                            rhs=expT[:, ip * 512 : (ip + 1) * 512],
                            start=(jc == 0),
                            stop=(jc == NJC - 1),
                            skip_group_check=True,
                        )

                # ---- Normalize: out[d,i] = acc[d,i] * (1 / s[i])
                sums = norm_pool.tile([1, N], f32, tag="sums")
                for ip in range(NIP):
                    nc.vector.tensor_copy(
                        out=sums[:, ip * 512 : (ip + 1) * 512],
                        in_=av[ip][D : D + 1, :],
                    )
                recip = norm_pool.tile([1, N], f32, tag="recip")
                nc.vector.reciprocal(out=recip, in_=sums)
                # SBUF APs cannot partition-broadcast in DMA; bounce through
                # DRAM, whose APs can (step-0 partition dim on the read).
                recip_dr = dram_pool.tile([1, N], f32, tag="recip_dr")
                nc.sync.dma_start(out=recip_dr, in_=recip)
                recip_bc = norm_pool.tile([D, N], f32, tag="rbc")
                nc.sync.dma_start(out=recip_bc, in_=recip_dr.to_broadcast([D, N]))
                out_sb = out_pool.tile([D, N], f32, tag="out")
                for ip in range(NIP):
                    nc.vector.tensor_mul(
                        out=out_sb[:, ip * 512 : (ip + 1) * 512],
                        in0=av[ip][0:D, :],
                        in1=recip_bc[:, ip * 512 : (ip + 1) * 512],
                    )
                nc.sync.dma_start(out=out_d[h], in_=out_sb)

        const_pool.release()

    nc.finalize()
    return nc


_NC_CACHE = None


def _get_nc():
    global _NC_CACHE
    if _NC_CACHE is None:
        _NC_CACHE = _build_bass()
    return _NC_CACHE


def kernel(q, k, v, _trace=False):
    from concourse.bass_utils import run_bass_kernel_spmd

    qf = np.ascontiguousarray(np.asarray(q, dtype=np.float32).reshape(B * H, D, N))
    kf = np.ascontiguousarray(np.asarray(k, dtype=np.float32).reshape(B * H, D, N))
    vf = np.ascontiguousarray(np.asarray(v, dtype=np.float32).reshape(B * H, D, N))

    in_maps = [
        {
            "q": qf[c * HPC : (c + 1) * HPC],
            "k": kf[c * HPC : (c + 1) * HPC],
            "v": vf[c * HPC : (c + 1) * HPC],
        }
        for c in range(NCORES)
    ]

    nc = _get_nc()
    res = run_bass_kernel_spmd(nc, in_maps, list(range(NCORES)), trace=_trace)
    out = np.concatenate([res.results[c]["out"] for c in range(NCORES)], axis=0)
    if _trace:
        kernel.last_exec_time_ns = res.exec_time_ns
        kernel.last_mean_exec_time_ns = res.mean_exec_time_ns
    return out.reshape(B, H, D, N).astype(np.float32, copy=False)
